# revision 4
# baseline (speedup 1.0000x reference)
"""Trainium2 Bass kernel for nn_ConvQuantizationWrapper.

The reference bit-slices an 8-bit quantized 3x3 conv into 32 (2-bit act x
1-bit weight) conv passes and recombines them with powers of two. That
decomposition exactly reconstructs

    out = conv2d(A, Wq) / (sa*sw) + bias
    A   = clip(round(x*sa - zp), 0, 255) + zp        (integers in [-128,127])
    Wq  = wrap_int8(round(w*sw))                     (integers in [-128,127])

in exact integer arithmetic (all partial sums < 2^24, so f32/bf16-input
matmuls are exact). The kernel therefore runs one quantized conv:

  - data-parallel over batch: 8 images per NeuronCore
  - per image pair: quantize on DVE (2 tensor_scalar ops; round via the
    +1.5*2^23 magic-number trick, replicating the reference's f32 rounding
    bit-exactly), bf16 result written into a zero-padded [58,58] layout
  - partition-swap copy so each image exists in both partition halves
  - 3x3 conv = 9 shifted [64,64] bf16 matmuls accumulated in PSUM,
    issued as 4 concurrent PE sub-tiles (2 row groups x 2 col groups =
    two images x two tap row-groups) for full 128x128 array utilization
  - epilogue on ACT: out = psum * (1/(sa*sw)) + bias  (per-channel bias)
"""

import numpy as np
import ml_dtypes

import concourse.bacc as bacc
import concourse.mybir as mybir
import concourse.tile as tile
from concourse import bass_utils

N_CORES = 8
IMGS = 8          # images per core (batch 64 / 8 cores)
C = 64
H = W = 56
HP = 58           # padded spatial
NPIX = H * W      # 3136
CHUNK_ROWS = 8
CHUNK = CHUNK_ROWS * W   # 448 output pixels per PSUM bank
NCHUNKS = H // CHUNK_ROWS
MAGIC = 12582912.0       # 1.5 * 2**23: float32 round-to-nearest-integer trick

_nc_cache = {}


def _build(sa: float, neg_zp: float, recip: float, reps: int = 1):
    """Build + compile the per-core Bass kernel (cached per scalar config)."""
    key = (sa, neg_zp, recip, reps)
    if key in _nc_cache:
        return _nc_cache[key]

    A = mybir.AluOpType
    nc = bacc.Bacc("TRN2", target_bir_lowering=False, debug=False)
    x_d = nc.dram_tensor("x", [IMGS, C, H, W], mybir.dt.float32,
                         kind="ExternalInput").ap()
    w_d = nc.dram_tensor("wt", [128, 9 * 64], mybir.dt.bfloat16,
                         kind="ExternalInput").ap()
    b_d = nc.dram_tensor("biasd", [128, 1], mybir.dt.float32,
                         kind="ExternalInput").ap()
    y_d = nc.dram_tensor("y", [IMGS, C, H, W], mybir.dt.float32,
                         kind="ExternalOutput").ap()

    taps = [(kh - 1, kw - 1) for kh in range(3) for kw in range(3)]

    with tile.TileContext(nc) as tc:
        with (
            tc.tile_pool(name="const", bufs=1) as cpool,
            tc.tile_pool(name="xbuf", bufs=1) as xpool,
            tc.tile_pool(name="work", bufs=2) as wpool,
            tc.tile_pool(name="psum", bufs=8, space="PSUM") as ppool,
        ):
            w_sb = cpool.tile([128, 9 * 64], mybir.dt.bfloat16, name="w_sb")
            nc.sync.dma_start(out=w_sb, in_=w_d)
            b_sb = cpool.tile([128, 1], mybir.dt.float32, name="b_sb")
            nc.sync.dma_start(out=b_sb, in_=b_d)

            # Persistent double-buffered quantized-image tiles.
            # X1 = [imgP on parts 0-63 ; imgQ on parts 64-127], X2 = swapped.
            Xbufs = []
            for j in range(2):
                X1 = xpool.tile([128, HP, HP], mybir.dt.bfloat16,
                                name=f"X1_{j}", tag=f"X1_{j}")
                X2 = xpool.tile([128, HP, HP], mybir.dt.bfloat16,
                                name=f"X2_{j}", tag=f"X2_{j}")
                # zero once: interior is rewritten every pair, border stays 0
                nc.vector.memset(X1.rearrange("p a b -> p (a b)"), 0.0)
                Xbufs.append((X1, X2))

            for rep in range(reps):
              for pair in range(IMGS // 2):
                X1, X2 = Xbufs[pair % 2]
                xf = wpool.tile([128, NPIX], mybir.dt.float32,
                                name="xf", tag="xf")
                nc.sync.dma_start(
                    out=xf,
                    in_=x_d[2 * pair:2 * pair + 2].rearrange(
                        "i c h w -> (i c) (h w)"))

                # t1 = (x * sa) + (-zp)   -- two chained f32 ALU ops, same
                # rounding sequence as the reference's x*sa - zp
                t1 = wpool.tile([128, NPIX], mybir.dt.float32,
                                name="t1", tag="t1")
                nc.vector.tensor_scalar(t1, xf, sa, neg_zp,
                                        op0=A.mult, op1=A.add)
                # A = round(t1) + zp  ->  bf16 into padded interior
                # (t1 + MAGIC) rounds to integer (RNE); subtract MAGIC+(-zp)
                nc.vector.tensor_scalar(
                    X1[:, 1:57, 1:57],
                    t1.rearrange("p (h w) -> p h w", h=H),
                    MAGIC, MAGIC + neg_zp,
                    op0=A.add, op1=A.subtract)
                # partition-swap duplicate (borders copied along -> zeros)
                nc.vector.tensor_copy(
                    X2[64:128].rearrange("p a b -> p (a b)"),
                    X1[0:64].rearrange("p a b -> p (a b)"))
                nc.vector.tensor_copy(
                    X2[0:64].rearrange("p a b -> p (a b)"),
                    X1[64:128].rearrange("p a b -> p (a b)"))

                ystage = wpool.tile([128, NPIX], mybir.dt.float32,
                                    name="ystage", tag="ystage")
                for ch in range(NCHUNKS):
                    ps = ppool.tile([128, CHUNK], mybir.dt.float32,
                                    name="ps", tag="ps")
                    # one row-group per accumulation group (HW requirement:
                    # mixed row-groups / interleaved groups in one bank
                    # hang); alternate per chunk+pair for 4-quadrant balance
                    rg = 64 * ((ch + pair) % 2)
                    bufP = X1 if rg == 0 else X2
                    bufQ = X2 if rg == 0 else X1
                    for half, buf in ((0, bufP), (64, bufQ)):
                        for t in range(9):
                            dh, dw = taps[t]
                            rs = CHUNK_ROWS * ch + 1 + dh
                            cs = 1 + dw
                            lhsT = w_sb[rg:rg + 64, t * 64:(t + 1) * 64]
                            mov = buf[rg:rg + 64, rs:rs + 8, cs:cs + 56]
                            nc.tensor.matmul(ps[half:half + 64], lhsT, mov,
                                             start=(t == 0), stop=(t == 8))
                    # epilogue: y = psum * recip + bias (per-partition)
                    nc.scalar.activation(
                        out=ystage[:, ch * CHUNK:(ch + 1) * CHUNK],
                        in_=ps,
                        func=mybir.ActivationFunctionType.Identity,
                        bias=b_sb, scale=recip)

                nc.sync.dma_start(
                    out=y_d[2 * pair:2 * pair + 2].rearrange(
                        "i c h w -> (i c) (h w)"),
                    in_=ystage)

    nc.compile()
    _nc_cache[key] = nc
    return nc


def _prep(x, weight, bias, scale_a, scale_w, zero_point):
    x = np.ascontiguousarray(np.asarray(x, dtype=np.float32))
    weight = np.asarray(weight, dtype=np.float32)
    bias = np.asarray(bias, dtype=np.float32)
    sa = float(np.asarray(scale_a).reshape(-1)[0])
    sw = float(np.asarray(scale_w).reshape(-1)[0])
    zp = float(np.asarray(zero_point).reshape(-1)[0])

    # activation-clip guard: reference clips round(x*sa - zp) to [0, 255].
    # For in-range data the clip is a no-op; if any value could clip,
    # pre-clamp x on the host (preserves the reference's semantics).
    amax = float(np.abs(x).max())
    if not (amax * abs(sa) < abs(zp if zp != 0 else 0) + 126.99 and
            -0.49 < -zp and sa * amax - zp < 255.49):
        f32 = np.float32
        lo = (f32(-0.49) + f32(zp)) / f32(sa)
        hi = (f32(255.49) + f32(zp)) / f32(sa)
        x = np.clip(x, lo, hi).astype(np.float32)

    # weight quantization, matching jnp.round(weight * sw) in f32 + the
    # implicit 8-bit two's-complement wrap of the bit decomposition
    qw = np.round(weight * np.float32(sw))
    qwi = qw.astype(np.int64)
    qw_eff = ((qwi + 128) % 256) - 128
    delta = qwi - qw_eff          # nonzero only if |qw| > 127 (never for
    # randn*20 weights); handled via a host-side correction plane below.

    wt = qw_eff.astype(np.float32).transpose(1, 2, 3, 0).reshape(C, 9 * C)
    wt_dup = np.ascontiguousarray(
        np.concatenate([wt, wt], axis=0)).astype(ml_dtypes.bfloat16)
    bias_dup = np.ascontiguousarray(
        np.concatenate([bias, bias])[:, None].astype(np.float32))

    sprod = np.float32(sw) * np.float32(sa)
    recip = float(np.float32(1.0) / sprod)

    corr = None
    if np.any(delta != 0):
        # reference's zero-point term uses the unwrapped qw:
        # out_ref - out_dev = zp * conv2d(ones, delta) * recip
        dsum = delta.sum(axis=1).astype(np.float64)  # [o, 3, 3]
        plane = np.zeros((C, H, W), np.float64)
        for kh in range(3):
            for kw in range(3):
                h0, h1 = max(0, 1 - kh), min(H, H + 1 - kh)
                w0, w1 = max(0, 1 - kw), min(W, W + 1 - kw)
                plane[:, h0:h1, w0:w1] += dsum[:, kh, kw][:, None, None]
        corr = (zp * plane * float(recip)).astype(np.float32)

    return x, wt_dup, bias_dup, sa, zp, recip, corr


def _run(x, weight, bias, scale_a, scale_w, zero_point, trace=False):
    x, wt_dup, bias_dup, sa, zp, recip, corr = _prep(
        x, weight, bias, scale_a, scale_w, zero_point)
    nc = _build(sa, -zp, recip)
    n = x.shape[0]
    assert n == N_CORES * IMGS, f"expected batch {N_CORES * IMGS}, got {n}"
    in_maps = [
        {"x": np.ascontiguousarray(x[k * IMGS:(k + 1) * IMGS]),
         "wt": wt_dup, "biasd": bias_dup}
        for k in range(N_CORES)
    ]
    res = bass_utils.run_bass_kernel_spmd(
        nc, in_maps, core_ids=list(range(N_CORES)), trace=trace)
    y = np.concatenate([res.results[k]["y"] for k in range(N_CORES)], axis=0)
    if corr is not None:
        y = y + corr[None]
    return np.ascontiguousarray(y.astype(np.float32)), res


def kernel(x, weight, bias, scale_a, scale_w, zero_point):
    y, _ = _run(x, weight, bias, scale_a, scale_w, zero_point, trace=False)
    return y
